# revision 16
# baseline (speedup 1.0000x reference)
"""Block-diagonal GRU cell for Trainium2, data-parallel over 8 NeuronCores.

Math (per batch row b, block j of 8, block size 256):
    wx  = x @ W_ir.T + b_ir_lin + b_ir          # [B, 6144], gates r|z|n global-chunked
    wh  = hb_j @ W_h[j].T + b_hr_j              # per block, local r|z|n chunks of 256
    r   = sigmoid(wxr + whr)
    z   = sigmoid(wxz + whz)
    n   = tanh(wxn + r * whn)
    h'  = (1-z)*hb + z*n

Device strategy (per core, batch-sharded 1024 rows):
  - Mixed fp8/fp16 matmuls, fp8e4m3 with MatmulPerfMode.DoubleRow (2
    contraction rows per instruction; all matmuls stream 1 output
    col/cycle, so DR = 2x throughput per contraction depth). The r/z
    gates (x- and h-proj) and the n-gate h-proj run fp8 DR. The n-gate
    x-projection is split-K mixed precision: leading 6 of 8 k-chunks
    fp8 DR + trailing 2 chunks fp16 (fp8 error scales as sqrt(fraction):
    all-fp16-n sims 1.2446e-2, all-fp8 2.094e-2, this 6/8 split
    1.916996e-2 vs the 2e-2 gate; the numpy sim in the session notes
    matches HW to 6 digits, seed-deterministic). PE work: 262144
    cycles/core ~= 109us @ 2.4GHz vs 311296 for the fp16-n version.
  - fp8 weights are pre-scaled by S=64 on host (W~N(0,0.02) sits at the
    e4m3 subnormal boundary); 1/S folds into the ACT `scale` operand.
    fp16 n-gate weights get the same x64 so PSUM bank B is uniform.
  - PSUM per pair item (blocks 2-7 are pair-banked): A0/A1 accumulate
    64*(wxr+whr | wxz+whz) per block; BX holds 64*[wxn_a|wxn_b] (3 fp8
    DR + 2 fp16 chunks), BH holds 64*[whn_a|whn_b]. Blocks 0-1 run
    j-serial so the DMA-bound head only needs block 0 weights first.
  - Fused pair epilogue in fp16 (one [P,512] op per step -- DVE/ACT ops
    are ~250-450ns fixed overhead each, so two [P,256] halves waste
    ~40%): both SIGs write one [P, 2, 512] tile; strided views give
    [r_a|r_b] / [z_a|z_b] aligned with BH/h16:
        rz   = sigmoid(A/S)     ACT x2      t3 = r * BH        DVE
        t4   = BX + t3          DVE         n  = tanh(t4/S)    ACT
        d    = n - h16          DVE         t5 = z * d         DVE
        oj   = t5 + h16         DVE (keeps the chain on one engine;
                                     Pool is reserved for head loads)
    j-serial items keep per-block [P,256] epilogue with the final add
    on GpSimd. Output is stored fp16, upcast on host.
  - Epilogue is software-pipelined one item behind the matmuls (pend);
    for the last pair it flushes at the item TOP so the final chains
    overlap the last matmuls, and the final item runs as two j-serial
    halves (a half chain ~2.8us tail vs a pair chain ~5.7us).
  - DMA trigger issue (~625-705ns per HWDGE trigger on SP/ACT) is the
    ramp-phase limiter; three queues are used in need-time order:
      SP  : xt8[0], wrz8[0] k-pairs, ht streams, xt8[m], steady weights
      ACT : wn8[jp0] kp-split, xt16[*], out stores (batched [128,512]
            per (m,pair); stores must NOT ride SP -- the trigger waits
            on the epilogue ADD and blocks prefetches behind it)
      Pool: whrz8[0]/whn8[0]/wn16[0] + ramp h16 (SWDGE ~1us/trigger but
            the engine is idle pre-epilogue; mid-run Pool triggers queue
            behind dependency-waiting ADDs -- head-phase use only)
    Weight prefetches are emitted MID m-loop so they never queue ahead
    of ramp m-streams; keeping ACT light in the ramp matters because
    SIG releases PSUM bank A and queues behind ACT triggers.
  - Host layouts make every DMA contiguous >=512B per partition line;
    many small triggers beat one consolidated DMA (measured +4us).

Later refinements: ACT-table prewarm emitted FIRST (engine-side table
load overlaps the sequencer's D2D triggers; prewarm-after-triggers made
the first SIG wait on it ~13us in, stalling the PE via PSUM bank-A
pressure, ~2-3us); x-projections emitted one item AHEAD of each item
tail in the j-serial phase (PE has ~1.4us of queued x-work while the
tail's operands land during the DMA-bound ramp; holds 3 of 4 PSUM
banks); wrz8[0] k-pairs split across SP+ACT at the head; the last TWO
items run as j-serial halves with the final halves using a DVE add +
per-half eager [128,256] stores (post-matmul drain ~6->4.3us).

Measured on 8 trn2 cores: 136113/139927 ns fast-state samples
(MATMUL-512 avg ~228ns), rel err 1.916996e-2 (= sim prediction; fixed
seed makes this deterministic). The same NEFF lands bimodally at
fast/slow chip clock states (~18%, not program-caused): best
slow-state sample 164159 ns @ ~273ns avg. History: first split-K
version 142426 fast; fp16-n baseline 161511 fast / ~193k slow; bf16
baseline 241842. PE busy ~113.5us (262144 cycles, the error-budget
floor) with <1us of micro-gaps; fixed ~9.6us post-store runtime
postamble (Q_XIV drain) is present in every measurement -- not
optimizable.
"""

import sys

if "/opt/trn_rl_repo" not in sys.path:
    sys.path.insert(0, "/opt/trn_rl_repo")

import numpy as np

B, IN, H, NB = 8192, 1024, 2048, 8
BS = H // NB  # 256
NCORES = 8
BC = B // NCORES  # 1024 rows per core
P = 128
S = 64.0  # fp8 weight prescale
PIPELINE_EPILOGUE = True

_BUILD_CACHE = {}


def build_nc(bc=BC, has_bias=False):
    """Build the Bass program for one core (SPMD: same program on all 8)."""
    key = (bc, has_bias)
    if key in _BUILD_CACHE:
        return _BUILD_CACHE[key]

    from contextlib import ExitStack

    import concourse.bacc as bacc
    import concourse.mybir as mybir
    import concourse.tile as tile

    f8 = mybir.dt.float8e4
    f16 = mybir.dt.float16
    f32 = mybir.dt.float32
    SIG = mybir.ActivationFunctionType.Sigmoid
    TANH = mybir.ActivationFunctionType.Tanh
    DR = mybir.MatmulPerfMode.DoubleRow

    K1 = IN // P  # 8 contraction chunks for the x projection
    K2 = BS // P  # 2 contraction chunks per block for the h projection
    K8P = 3  # n x-proj: leading 3 DR k-pairs (6 chunks) in fp8
    K16 = K1 - 2 * K8P  # trailing 2 chunks in fp16 (error-critical tail)
    MT = bc // P  # m-tiles (128 batch rows each)
    NJP = NB // 2  # block pairs

    nc = bacc.Bacc(target_bir_lowering=False)

    xt8_d = nc.dram_tensor("xt8", [P, MT, K1, P], f8, kind="ExternalInput").ap()
    xt16_d = nc.dram_tensor("xt16", [P, MT, K16, P], f16, kind="ExternalInput").ap()
    ht8_d = nc.dram_tensor("ht8", [P, MT, NJP, 2 * K2, P], f8, kind="ExternalInput").ap()
    h16_d = nc.dram_tensor("h16", [bc, H], f16, kind="ExternalInput").ap()
    wrz8_d = nc.dram_tensor("wrz8", [P, NB, K1, 2 * BS], f8, kind="ExternalInput").ap()
    wn8_d = nc.dram_tensor(
        "wn8", [P, NJP, K8P, 2, 2 * BS], f8, kind="ExternalInput"
    ).ap()
    wn16_d = nc.dram_tensor("wn16", [P, NB, K16, BS], f16, kind="ExternalInput").ap()
    whrz8_d = nc.dram_tensor("whrz8", [P, K2, NB, 2 * BS], f8, kind="ExternalInput").ap()
    whn8_d = nc.dram_tensor("whn8", [P, K2, NB, BS], f8, kind="ExternalInput").ap()
    if has_bias:
        brz_d = nc.dram_tensor("brz", [1, NB * 2 * BS], f32, kind="ExternalInput").ap()
        bxn_d = nc.dram_tensor("bxn", [1, NB * BS], f32, kind="ExternalInput").ap()
        bhn_d = nc.dram_tensor("bhn", [1, NB * BS], f32, kind="ExternalInput").ap()
    out = nc.dram_tensor("out", [bc, H], f16, kind="ExternalOutput").ap()

    with tile.TileContext(nc) as tc, ExitStack() as ctx:
        wpool = ctx.enter_context(tc.tile_pool(name="wres", bufs=1))
        spool = ctx.enter_context(tc.tile_pool(name="stream", bufs=MT + MT // 2))
        psA = ctx.enter_context(tc.tile_pool(name="psA", bufs=4, space="PSUM"))
        psB = ctx.enter_context(tc.tile_pool(name="psB", bufs=4, space="PSUM"))
        epool = ctx.enter_context(tc.tile_pool(name="epi", bufs=6))
        opool = ctx.enter_context(tc.tile_pool(name="ostage", bufs=MT + 2))

        # ---- resident tiles ----
        xt8_sb = wpool.tile([P, MT, K1, P], f8, tag="xt8_sb")
        xt16_sb = wpool.tile([P, MT, K16, P], f16, tag="xt16_sb")
        wrz8_sb = wpool.tile([P, NB, K1, 2 * BS], f8, tag="wrz8_sb")
        wn8_sb = wpool.tile([P, NJP, K8P, 2, 2 * BS], f8, tag="wn8_sb")
        wn16_sb = wpool.tile([P, NB, K16, BS], f16, tag="wn16_sb")
        whrz8_sb = wpool.tile([P, K2, NB, 2 * BS], f8, tag="whrz8_sb")
        whn8_sb = wpool.tile([P, K2, NB, BS], f8, tag="whn8_sb")

        def load_w_cols(j):
            # per-block steady-state weight prefetch; wn8 is pair-major and
            # loads once per pair (on the even block). All on SP: the Pool
            # SWDGE queue measured unusable mid-run (triggers queue behind
            # dependency-waiting epilogue ADDs -> 7us PE stall at a block
            # boundary), and ACT carries the xt8/xt16/store traffic.
            nc.sync.dma_start(wrz8_sb[:, j], wrz8_d[:, j])
            if j % 2 == 0:
                nc.sync.dma_start(wn8_sb[:, j // 2], wn8_d[:, j // 2])
            nc.sync.dma_start(wn16_sb[:, j], wn16_d[:, j])
            nc.sync.dma_start(whrz8_sb[:, :, j], whrz8_d[:, :, j])
            nc.sync.dma_start(whn8_sb[:, :, j], whn8_d[:, :, j])

        def load_mp_streams(m, jp):
            # per-(m, pair) small loads: many small triggers beat few big
            # DMAs here — one trigger's descriptors stay on one DMA engine,
            # so big consolidated transfers serialize (measured +4us).
            # ht rides SP; h16 (epilogue-only data) rides the DVE HWDGE
            # ring, thinning the SP trigger queue in the ramp phase.
            ht_mp = spool.tile([P, 2 * K2, P], f8, tag="ht_mp")
            nc.sync.dma_start(ht_mp[:], ht8_d[:, m, jp])
            h_mp = spool.tile([P, 2 * BS], f16, tag="h_mp")
            msl = slice(m * P, (m + 1) * P)
            psl = slice(2 * jp * BS, (2 * jp + 2) * BS)
            if jp == 0 and m < 4:
                # ramp phase: h16 (epilogue-only, slack-rich) rides the Pool
                # SWDGE queue (~1us/trigger but idle engine), thinning the
                # trigger-rate-limited SP queue
                nc.gpsimd.dma_start(h_mp[:], h16_d[msl, psl])
            else:
                nc.sync.dma_start(h_mp[:], h16_d[msl, psl])
            return ht_mp, h_mp

        # head: 3-way trigger-queue parallelism (~625ns/HWDGE trigger is
        # the ramp-phase limiter; DVE HWDGE exists in hw but bass only
        # exposes SP/ACT/Pool). Per-queue order matches the in-order PE
        # queue's need times for (j0, m0). The A-path k-pairs split
        # ACROSS SP and ACT so the first four matmuls' operands land via
        # two rings in parallel instead of serializing on one:
        #   SP  : xt8[0], wrz8[0] k2/k3 pairs, then m-streams
        #   ACT : wrz8[0] k0/k1 pairs, wn8[jp0] kp-split, xt16[1,5..7]
        #   Pool: xt16[0], wn16[0], whrz8[0], whn8[0] (item tail),
        #         ramp h16 + xt16[2..4] (all issued BEFORE the first
        #         epilogue ADD can queue on Pool)
        # PE p-state warmup: the Tensor engine ramps to full clock only
        # after ~3us of continuous execution (measured: first ~10us of
        # real matmuls averaged 502ns vs 216ns steady for 512-col). Burn
        # the ramp on dummy matmuls over a zeroed tile while the head
        # DMAs are still in flight; the dummy group is start/stop-closed
        # into one psA bank that real groups later overwrite (start=True
        # marks the bank pending-zero), so numerics are untouched.
        wu = wpool.tile([P, 4 * P], f8, tag="wu")
        nc.vector.memset(wu[:], 0.0)
        A_wu = psA.tile([P, 2 * BS], f32, tag="A")
        NWU = 8  # ~3us of ramping dummies; 11 measured worse (the extra
        # dummies delay real work past first-data without fixing the
        # later item-1-3 stream waits)
        for wi in range(NWU):
            nc.tensor.matmul(
                A_wu[:, :],
                lhsT=wu[:, 0:P],
                rhs=wu[:, :],
                start=(wi == 0), stop=(wi == NWU - 1),
            )
        # prewarm the ACT table set (sigmoid_and_others contains Tanh
        # too): the ~2.6us engine-side table load runs while the ACT
        # sequencer keeps issuing D2D triggers behind it. When the prewarm
        # sat after 5 triggers, the table load finished ~13us in and the
        # first SIG (whose completion releases PSUM bank A0) waited on it
        # -- a measured ~1.4us PE stall at item 2-3 via bank pressure.
        ws = wpool.tile([P, 1], f32, tag="ws")
        nc.vector.memset(ws[:], 0.0)
        nc.scalar.activation(ws[:], ws[:], SIG)
        nc.sync.dma_start(xt8_sb[:, 0], xt8_d[:, 0])
        streams = {}
        for k in (0, 1):
            nc.scalar.dma_start(
                wrz8_sb[:, 0, 2 * k : 2 * k + 2], wrz8_d[:, 0, 2 * k : 2 * k + 2]
            )
        for k in (2, 3):
            nc.sync.dma_start(
                wrz8_sb[:, 0, 2 * k : 2 * k + 2], wrz8_d[:, 0, 2 * k : 2 * k + 2]
            )
        for kp in range(K8P):
            nc.scalar.dma_start(wn8_sb[:, 0, kp], wn8_d[:, 0, kp])
        # item-0's tail weights ride ACT behind the wn8 chunks: the Pool
        # SWDGE ring moves data ~4x slower than HWDGE (its transfers were
        # still in flight at 13-17us, gating items 0-3). Pool keeps only
        # the slack-rich xt16[0] / ramp h16 / xt16[2..4].
        nc.gpsimd.dma_start(xt16_sb[:, 0], xt16_d[:, 0])
        nc.scalar.dma_start(wn16_sb[:, 0], wn16_d[:, 0])
        nc.scalar.dma_start(whrz8_sb[:, :, 0], whrz8_d[:, :, 0])
        nc.scalar.dma_start(whn8_sb[:, :, 0], whn8_d[:, :, 0])
        streams[(0, 0)] = load_mp_streams(0, 0)
        for m in range(1, MT):
            # xt8 on SP (the ht prefetches have slack there); xt16 spread
            # over ACT (m=1, 5..7) and Pool (m=2..4 -- issued early enough
            # to clear before the first epilogue ADDs reach the Pool
            # queue). Keeping ACT light in the ramp matters because SIGs
            # queue behind ACT triggers and SIG(k) releases PSUM bank A(k).
            nc.sync.dma_start(xt8_sb[:, m], xt8_d[:, m])
            if 2 <= m <= 4:
                nc.gpsimd.dma_start(xt16_sb[:, m], xt16_d[:, m])
            else:
                nc.scalar.dma_start(xt16_sb[:, m], xt16_d[:, m])
            streams[(m, 0)] = load_mp_streams(m, 0)
        if has_bias:
            ones_sb = wpool.tile([1, P], f32, tag="ones_sb")
            nc.vector.memset(ones_sb[:], 1.0)
            brz_sb = wpool.tile([1, NB * 2 * BS], f32, tag="brz_sb")
            bxn_sb = wpool.tile([1, NB * BS], f32, tag="bxn_sb")
            bhn_sb = wpool.tile([1, NB * BS], f32, tag="bhn_sb")
            nc.sync.dma_start(brz_sb[:], brz_d[:])
            nc.sync.dma_start(bxn_sb[:], bxn_d[:])
            nc.sync.dma_start(bhn_sb[:], bhn_d[:])

        ostage = {}

        def finish_epilogue(j, m, rz, Bt, h_mp, oj2, last=False):
            jp, half_i = divmod(j, 2)
            half = slice(half_i * BS, (half_i + 1) * BS)
            msl = slice(m * P, (m + 1) * P)
            t3 = epool.tile([P, BS], f16, tag="t3")
            nc.vector.tensor_mul(t3[:], rz[:, 0:BS], Bt[:, BS : 2 * BS])
            t4 = epool.tile([P, BS], f16, tag="t4")
            nc.vector.tensor_add(t4[:], Bt[:, 0:BS], t3[:])
            tn = epool.tile([P, BS], f16, tag="tn")
            nc.scalar.activation(tn[:], t4[:], TANH, scale=1.0 / S)
            d = epool.tile([P, BS], f16, tag="d")
            nc.vector.tensor_sub(d[:], tn[:], h_mp[:, half])
            t5 = epool.tile([P, BS], f16, tag="t5")
            nc.vector.tensor_mul(t5[:], rz[:, BS : 2 * BS], d[:])
            if last:
                # drain-tail halves: the final add stays on DVE (no
                # engine hop after t5) and each half stores [128,256]
                # immediately instead of waiting for the batched pair
                # store -- shaves ~1us off the post-matmul drain
                nc.vector.tensor_add(oj2[:, half], t5[:], h_mp[:, half])
                hsl = slice((2 * jp + half_i) * BS, (2 * jp + half_i + 1) * BS)
                nc.scalar.dma_start(out[msl, hsl], oj2[:, half])
            else:
                nc.gpsimd.tensor_add(oj2[:, half], t5[:], h_mp[:, half])
                if half_i == 1:
                    # one batched [128, 512] fp16 store per (m, block-pair)
                    # on the ACT HWDGE ring. (On SP the trigger's wait on
                    # the gpsimd ADD blocks the prefetches queued behind it
                    # -- measured ~1-2us PE gaps through the pair phase.
                    # gpsimd DMAs are software-DGE: also avoid.)
                    psl = slice(2 * jp * BS, (2 * jp + 2) * BS)
                    nc.scalar.dma_start(out[msl, psl], oj2[:, :])
            if half_i == 1:
                del ostage[m]
                streams.pop((m, jp, "cur"))
                # this m's pair tiles just released: prefetch its
                # next-pair streams now
                if jp + 1 < NJP:
                    streams[(m, jp + 1)] = load_mp_streams(m, jp + 1)

        pend = [None]

        def emit_xprojs(j, m):
            A = psA.tile([P, 2 * BS], f32, tag="A")
            Bt = psB.tile([P, 2 * BS], f32, tag="B")
            # rz x-projection: fp8 DoubleRow over k-chunk pairs; B's
            # group start marks the whole bank pending-zero so the
            # h-side MMs overwrite-then-accumulate correctly.
            for k in range(K1 // 2):
                nc.tensor.matmul(
                    A[:, :],
                    lhsT=xt8_sb[:, m, 2 * k : 2 * k + 2, :],
                    rhs=wrz8_sb[:, j, 2 * k : 2 * k + 2, :],
                    start=(k == 0), stop=False, perf_mode=DR,
                )
            # n x-projection, split-K mixed precision: leading 6 k-chunks
            # fp8 DoubleRow, trailing 2 chunks fp16 (sim: 1.917e-2 total
            # vs 2.094e-2 all-fp8 / 1.245e-2 all-fp16-n, gate 2e-2)
            jp, half_i = divmod(j, 2)
            nsl = slice(half_i * BS, (half_i + 1) * BS)
            for kp in range(K8P):
                nc.tensor.matmul(
                    Bt[:, 0:BS],
                    lhsT=xt8_sb[:, m, 2 * kp : 2 * kp + 2, :],
                    rhs=wn8_sb[:, jp, kp, :, nsl],
                    start=(kp == 0), stop=False, perf_mode=DR,
                )
            for k in range(K16):
                nc.tensor.matmul(
                    Bt[:, 0:BS],
                    lhsT=xt16_sb[:, m, k, :],
                    rhs=wn16_sb[:, j, k, :],
                    start=False, stop=False,
                )
            return A, Bt

        def emit_rest(j, m, A, Bt, tail=False):
            jp, half_i = divmod(j, 2)
            jrz = slice(j * 2 * BS, (j + 1) * 2 * BS)
            jn = slice(j * BS, (j + 1) * BS)
            if half_i == 0:
                streams[(m, jp, "cur")] = streams.pop((m, jp))
                ostage[m] = opool.tile(
                    [P, 2 * BS], f16, tag="oj2", name=f"oj2_{m}"
                )
            ht_mp, h_mp = streams[(m, jp, "cur")]
            # block-diagonal h-projections: one fp8 DoubleRow each
            last = not has_bias
            nc.tensor.matmul(
                A[:, :],
                lhsT=ht_mp[:, 2 * half_i : 2 * half_i + 2, :],
                rhs=whrz8_sb[:, :, j, :],
                start=False, stop=last, perf_mode=DR,
            )
            nc.tensor.matmul(
                Bt[:, BS : 2 * BS],
                lhsT=ht_mp[:, 2 * half_i : 2 * half_i + 2, :],
                rhs=whn8_sb[:, :, j, :],
                start=False, stop=last, perf_mode=DR,
            )
            if has_bias:
                nc.tensor.matmul(
                    A[:, :], lhsT=ones_sb[:, :], rhs=brz_sb[:, jrz],
                    start=False, stop=True,
                )
                nc.tensor.matmul(
                    Bt[:, 0:BS], lhsT=ones_sb[:, :], rhs=bxn_sb[:, jn],
                    start=False, stop=False,
                )
                nc.tensor.matmul(
                    Bt[:, BS : 2 * BS], lhsT=ones_sb[:, :], rhs=bhn_sb[:, jn],
                    start=False, stop=True,
                )

            rz = epool.tile([P, 2 * BS], f16, tag="rz")
            nc.scalar.activation(rz[:], A[:, :], SIG, scale=1.0 / S)
            # epilogue is software-pipelined one (j,m) item behind the
            # matmuls: the in-order ACT queue would otherwise serialize
            # on the rz -> DVE t3/t4 -> tanh roundtrip (~2us, the same
            # as the PE's per-item rate) and accumulate a drain tail.
            if PIPELINE_EPILOGUE:
                if pend[0] is not None:
                    fn, args = pend[0]
                    fn(*args)
                pend[0] = (
                    finish_epilogue, (j, m, rz, Bt, h_mp, ostage[m], tail)
                )
            else:
                finish_epilogue(j, m, rz, Bt, h_mp, ostage[m], tail)

        def finish_pair(jp, m, rz2, BX, BH, h_mp, oj2):
            # fused pair epilogue: one [P,512] op per step instead of two
            # [P,256] halves (DVE/ACT ops are ~250-450ns fixed overhead
            # each). rz2 is [P, 2(block), 2BS] = [r_a|z_a ; r_b|z_b]; the
            # strided views below read [r_a|r_b] / [z_a|z_b], matching
            # BH = [whn_a|whn_b] and h_mp = [P, 512]. The final add runs
            # on DVE (chain stays on one engine; frees Pool entirely).
            msl = slice(m * P, (m + 1) * P)
            rv = rz2[:, :, 0:BS]
            zv = rz2[:, :, BS : 2 * BS]
            t3 = epool.tile([P, 2 * BS], f16, tag="t3")
            nc.vector.tensor_mul(t3[:], rv, BH[:, :])
            t4 = epool.tile([P, 2 * BS], f16, tag="t4")
            nc.vector.tensor_add(t4[:], BX[:, :], t3[:])
            tn = epool.tile([P, 2 * BS], f16, tag="tn")
            nc.scalar.activation(tn[:], t4[:], TANH, scale=1.0 / S)
            d = epool.tile([P, 2 * BS], f16, tag="d")
            nc.vector.tensor_sub(d[:], tn[:], h_mp[:])
            t5 = epool.tile([P, 2 * BS], f16, tag="t5")
            nc.vector.tensor_mul(t5[:], zv, d[:])
            nc.vector.tensor_add(oj2[:, :], t5[:], h_mp[:])
            psl = slice(2 * jp * BS, (2 * jp + 2) * BS)
            nc.scalar.dma_start(out[msl, psl], oj2[:, :])
            if jp + 1 < NJP:
                streams[(m, jp + 1)] = load_mp_streams(m, jp + 1)

        def emit_pair_item(jp, m, last=False):
            # pair-banked item: both blocks of the pair share one BX bank
            # (wxn side by side) and one BH bank, halving that instruction
            # class's dispatch and ldweights count
            ja, jb = 2 * jp, 2 * jp + 1
            ht_mp, h_mp = streams.pop((m, jp))
            if jp == NJP - 1 and pend[0] is not None:
                # last pair: flush the deferred epilogue BEFORE this item's
                # matmuls/activations so the final chains overlap the last
                # matmuls instead of serializing after them
                fn, args = pend[0]
                fn(*args)
                pend[0] = None
            A0 = psA.tile([P, 2 * BS], f32, tag="A")
            A1 = psA.tile([P, 2 * BS], f32, tag="A")
            BX = psB.tile([P, 2 * BS], f32, tag="B")
            BH = psB.tile([P, 2 * BS], f32, tag="B")
            for k in range(K1 // 2):
                nc.tensor.matmul(
                    A0[:, :],
                    lhsT=xt8_sb[:, m, 2 * k : 2 * k + 2, :],
                    rhs=wrz8_sb[:, ja, 2 * k : 2 * k + 2, :],
                    start=(k == 0), stop=False, perf_mode=DR,
                )
            for k in range(K1 // 2):
                nc.tensor.matmul(
                    A1[:, :],
                    lhsT=xt8_sb[:, m, 2 * k : 2 * k + 2, :],
                    rhs=wrz8_sb[:, jb, 2 * k : 2 * k + 2, :],
                    start=(k == 0), stop=False, perf_mode=DR,
                )
            for kp in range(K8P):
                nc.tensor.matmul(
                    BX[:, :],
                    lhsT=xt8_sb[:, m, 2 * kp : 2 * kp + 2, :],
                    rhs=wn8_sb[:, jp, kp, :, :],
                    start=(kp == 0), stop=False, perf_mode=DR,
                )
            for k in range(K16):
                nc.tensor.matmul(
                    BX[:, :],
                    lhsT=xt16_sb[:, m, k, :],
                    rhs=wn16_sb[:, ja : ja + 2, k, :],
                    start=False,
                    stop=(k == K16 - 1 and not has_bias),
                )
            stopf = not has_bias
            nc.tensor.matmul(
                A0[:, :], lhsT=ht_mp[:, 0:2, :], rhs=whrz8_sb[:, :, ja, :],
                start=False, stop=stopf, perf_mode=DR,
            )
            nc.tensor.matmul(
                A1[:, :], lhsT=ht_mp[:, 2:4, :], rhs=whrz8_sb[:, :, jb, :],
                start=False, stop=stopf, perf_mode=DR,
            )
            nc.tensor.matmul(
                BH[:, 0:BS], lhsT=ht_mp[:, 0:2, :], rhs=whn8_sb[:, :, ja, :],
                start=True, stop=False, perf_mode=DR,
            )
            nc.tensor.matmul(
                BH[:, BS : 2 * BS], lhsT=ht_mp[:, 2:4, :], rhs=whn8_sb[:, :, jb, :],
                start=False, stop=stopf, perf_mode=DR,
            )
            if has_bias:
                bsl = slice(ja * BS, (ja + 2) * BS)
                nc.tensor.matmul(
                    A0[:, :], lhsT=ones_sb[:, :],
                    rhs=brz_sb[:, ja * 2 * BS : (ja + 1) * 2 * BS],
                    start=False, stop=True,
                )
                nc.tensor.matmul(
                    A1[:, :], lhsT=ones_sb[:, :],
                    rhs=brz_sb[:, jb * 2 * BS : (jb + 1) * 2 * BS],
                    start=False, stop=True,
                )
                nc.tensor.matmul(
                    BX[:, :], lhsT=ones_sb[:, :], rhs=bxn_sb[:, bsl],
                    start=False, stop=True,
                )
                nc.tensor.matmul(
                    BH[:, :], lhsT=ones_sb[:, :], rhs=bhn_sb[:, bsl],
                    start=False, stop=True,
                )
            rz2 = epool.tile([P, 2, 2 * BS], f16, tag="rz")
            nc.scalar.activation(rz2[:, 0, :], A0[:, :], SIG, scale=1.0 / S)
            nc.scalar.activation(rz2[:, 1, :], A1[:, :], SIG, scale=1.0 / S)
            oj2 = opool.tile([P, 2 * BS], f16, tag="oj2", name=f"oj2p_{jp}_{m}")
            if last:
                finish_pair(jp, m, rz2, BX, BH, h_mp, oj2)
            else:
                if pend[0] is not None:
                    fn, args = pend[0]
                    fn(*args)
                pend[0] = (finish_pair, (jp, m, rz2, BX, BH, h_mp, oj2))

        # blocks 0-1 run j-serial (the DMA-bandwidth-bound head depends on
        # only block 0's weights being needed first); pairs 1-3 run
        # pair-banked. Weight prefetches are emitted MID m-loop so their
        # SP triggers don't queue ahead of the ramp-phase m-streams.
        # The x-projections are emitted one item AHEAD of the item tail
        # (h-proj + epilogue): the in-order PE then has ~1.4us of x-work
        # queued whenever the tail's operands (whrz/whn/wn16/streams) are
        # still in flight during the DMA-bound ramp. Holds 3 of 4 psA/psB
        # banks (ahead + current + pipelined-epilogue reader).
        xp = {}
        items = [(j, m) for j in range(2) for m in range(MT)]
        for idx, (j, m) in enumerate(items):
            if j == 0 and m == 3:
                load_w_cols(1)
            elif j == 1 and m == 1:
                load_w_cols(2)
            elif j == 1 and m == 3:
                load_w_cols(3)
            if (j, m) not in xp:
                xp[(j, m)] = emit_xprojs(j, m)
            if idx + 1 < len(items):
                nj, nm = items[idx + 1]
                if (nj, nm) not in xp:
                    xp[(nj, nm)] = emit_xprojs(nj, nm)
            A, Bt = xp.pop((j, m))
            emit_rest(j, m, A, Bt)
        for jp in range(1, NJP):
            for m in range(MT):
                if m in (1, 4):
                    nj = 2 * jp + 2 + (0 if m == 1 else 1)
                    if nj < NB:
                        load_w_cols(nj)
                if jp == NJP - 1 and m >= MT - 2:
                    # the last TWO items run as j-serial halves: a half's
                    # tail-exposed epilogue chain (~2.8us) is well under a
                    # fused pair item's (~4us), each deferred chain flushes
                    # with matmul runway still ahead of it, and the final
                    # halves use the DVE add + per-half eager stores
                    if pend[0] is not None:
                        fn, args = pend[0]
                        fn(*args)
                        pend[0] = None
                    for j in (2 * jp, 2 * jp + 1):
                        A, Bt = emit_xprojs(j, m)
                        emit_rest(j, m, A, Bt, tail=(m == MT - 1))
                else:
                    emit_pair_item(jp, m)

        if pend[0] is not None:
            fn, args = pend[0]
            fn(*args)

    nc.compile()
    _BUILD_CACHE[key] = nc
    return nc


def prep_inputs(x, h, W_ir, b_ir_lin, b_ir, W_h, b_hr, ncores=NCORES):
    """Host-side reshaping/casting -> per-core in_maps + has_bias flag."""
    import ml_dtypes

    f8 = ml_dtypes.float8_e4m3

    x = np.asarray(x, dtype=np.float32)
    h = np.asarray(h, dtype=np.float32)
    W_ir = np.asarray(W_ir, dtype=np.float32)
    W_h = np.asarray(W_h, dtype=np.float32)
    b_ir_lin = np.asarray(b_ir_lin, dtype=np.float32)
    b_ir = np.asarray(b_ir, dtype=np.float32)
    b_hr = np.asarray(b_hr, dtype=np.float32)

    bc = x.shape[0] // ncores
    K1 = IN // P
    K2 = BS // P
    MT = bc // P
    NJP = NB // 2

    # weights: gate-and-block reordered, x64 prescale, contraction-dim-major,
    # laid out so each per-block DMA slice is contiguous per partition line
    Wr = W_ir[0:H].reshape(NB, BS, IN)
    Wz = W_ir[H : 2 * H].reshape(NB, BS, IN)
    Wn_ = W_ir[2 * H :].reshape(NB, BS, IN)
    Wrz = np.concatenate([Wr, Wz], axis=1)  # [NB, 512, IN]
    # [P, NB, K1, 2BS]: [p, j, k, f] = S * Wrz[j, f, k*128+p]
    wrz8 = np.ascontiguousarray(
        (Wrz * S).reshape(NB, 2 * BS, K1, P).transpose(3, 0, 2, 1)
    ).astype(f8)
    # n x-proj split-K: leading 6 k-chunks (IN rows 0:768) fp8 pair-major
    # [P, NJP, 3, 2, 2BS]; trailing 2 chunks (rows 768:1024) fp16
    K8P, K16 = 3, 2
    NJPw = NB // 2
    wn8 = np.ascontiguousarray(
        (Wn_[:, :, : 2 * K8P * P] * S)
        .reshape(NJPw, 2, BS, K8P, 2, P)
        .transpose(5, 0, 3, 4, 1, 2)
        .reshape(P, NJPw, K8P, 2, 2 * BS)
    ).astype(f8)
    wn16 = np.ascontiguousarray(
        (Wn_[:, :, 2 * K8P * P :] * S).reshape(NB, BS, K16, P).transpose(3, 0, 2, 1)
    ).astype(np.float16)
    # [P, K2, NB, cols] from W_h slices (contraction = within-block h index)
    whrz8 = np.ascontiguousarray(
        (W_h[:, 0 : 2 * BS, :] * S).reshape(NB, 2 * BS, K2, P).transpose(3, 2, 0, 1)
    ).astype(f8)
    whn8 = np.ascontiguousarray(
        (W_h[:, 2 * BS :, :] * S).reshape(NB, BS, K2, P).transpose(3, 2, 0, 1)
    ).astype(f8)

    bx = b_ir_lin + b_ir
    bh = b_hr.reshape(NB, 3 * BS)
    brz = np.concatenate(
        [
            bx[0:H].reshape(NB, BS) + bh[:, 0:BS],
            bx[H : 2 * H].reshape(NB, BS) + bh[:, BS : 2 * BS],
        ],
        axis=1,
    ).reshape(1, NB * 2 * BS)
    bxn = bx[2 * H :].reshape(1, NB * BS).copy()
    bhn = bh[:, 2 * BS :].reshape(1, NB * BS).copy()
    has_bias = bool(np.any(brz) or np.any(bxn) or np.any(bhn))

    in_maps = []
    for c in range(ncores):
        csl = slice(c * bc, (c + 1) * bc)
        xc = x[csl]  # [bc, IN]
        hc = h[csl]  # [bc, H]
        # xt [P, MT, K1, 128]: [p, m, k, col] = x[m*128+col, k*128+p]
        xT = xc.T.reshape(K1, P, MT, P).transpose(1, 2, 0, 3)
        xt8 = np.ascontiguousarray(xT).astype(f8)
        xt16 = np.ascontiguousarray(xT[:, :, 2 * K8P :]).astype(np.float16)
        # ht8 [P, MT, NJP, 2K2, 128]: [p,m,jp,kk,c] = h[m*128+c, jp*512+kk*128+p]
        hT = hc.T.reshape(NJP, 2 * K2, P, MT, P).transpose(2, 3, 0, 1, 4)
        ht8 = np.ascontiguousarray(hT).astype(f8)
        m = {
            "xt8": xt8,
            "xt16": xt16,
            "ht8": ht8,
            "h16": np.ascontiguousarray(hc).astype(np.float16),
            "wrz8": wrz8,
            "wn8": wn8,
            "wn16": wn16,
            "whrz8": whrz8,
            "whn8": whn8,
        }
        if has_bias:
            m["brz"] = (brz * S).astype(np.float32)
            m["bxn"] = (bxn * S).astype(np.float32)
            m["bhn"] = (bhn * S).astype(np.float32)
        in_maps.append(m)
    return in_maps, has_bias, bc


def kernel(x, h, W_ir, b_ir_lin, b_ir, W_h, b_hr):
    from concourse.bass_utils import run_bass_kernel_spmd

    in_maps, has_bias, bc = prep_inputs(x, h, W_ir, b_ir_lin, b_ir, W_h, b_hr)
    nc = build_nc(bc=bc, has_bias=has_bias)
    try:
        res = run_bass_kernel_spmd(nc, in_maps, list(range(NCORES)))
    except Exception:
        # transient NRT device errors have been observed once in ~10 runs;
        # a single retry reuses the compiled NEFF
        res = run_bass_kernel_spmd(nc, in_maps, list(range(NCORES)))
    return np.concatenate(
        [res.results[c]["out"].astype(np.float32) for c in range(NCORES)], axis=0
    )

